# revision 4
# baseline (speedup 1.0000x reference)
"""Trainium2 Bass kernel for GraphConvolution message passing.

Computation (reference):
    atom_h = BN1(X @ W1)                       # [N, 128]
    neigh  = BN2(atom_h[src] @ W2)             # [E, 128]
    bonds  = BN3(bond_features @ W3)           # [E, 128]
    agg    = segment_sum(neigh * bonds, dest)  # [N, 128]
    out    = atom_h + agg

Strategy: fold BN + W1@W2 on the host, pre-gather neighbor features per
edge (xg = (X@W12)[src]) and stream them.  Per core the device runs a
3-stage pipeline per 1024-edge window pair:
  1. bonds matmuls as ROW-TILED K=64 pairs (two tiles concurrently in PE
     rows 0-63 / 64-127, W3 duplicated in both halves),
  2. PSUM evacuation (ACT) + gate multiply (DVE, bf16 2x),
  3. scatter as COL-TILED M=64 pairs of FIXED one-hot pattern matmuls
     accumulating into PSUM.
The fixed scatter patterns are possible because the host assigns atoms to
window slots so every 64-slot window has the same degree-capacity profile
(high-degree atoms are split into chunks and re-merged on the host); this
removes all per-tile one-hot construction and index streams.
Zh (all bias/BN terms + atom_h) is added on the host after download.
"""

import numpy as np
import ml_dtypes

import concourse.bass as bass
import concourse.tile as tile
from concourse import bacc, mybir
from concourse.bass_utils import run_bass_kernel_spmd

BF16 = ml_dtypes.bfloat16
BN_EPS = 1e-3

N, E, F_ATOM, F_BOND, U = 100000, 800000, 128, 64, 128
NCORES = 8
KPW = 8            # tiles (of 128 edge slots) per window: capacity 1024

TRACE = False
LAST_RESULTS = None

_prog_cache = {}

TGX = 96           # tiles per xg/bf DMA group
OG = 16            # window-pairs per output DMA
FUSE_MOD = 1000    # effectively off: ACT evacuates every pair


def _build_program(wpc, profile):
    """profile: tuple of 128 slot capacities (desc), sum == 1024."""
    key = (wpc, profile)
    if key in _prog_cache:
        return _prog_cache[key]

    NT = wpc * KPW
    own = wpc * 128
    f32, bf16 = mybir.dt.float32, mybir.dt.bfloat16

    nc = bacc.Bacc("TRN2", target_bir_lowering=False, debug=False,
                   num_devices=NCORES)

    xgT = nc.dram_tensor("xgT", [128, NT * 128], bf16, kind="ExternalInput")
    # bf features packed in partition halves: rows 0-63 = tiles 0-3 of each
    # pair (A side), rows 64-127 = tiles 4-7 (B side)
    bfT = nc.dram_tensor("bfT", [128, (NT // 2) * 128], bf16,
                         kind="ExternalInput")
    # W3 duplicated into both partition halves
    w3 = nc.dram_tensor("w3", [128, 128], bf16, kind="ExternalInput")
    # 4 fixed one-hot patterns of [128 edge, 64 slot] (shared by A/B sides)
    pat = nc.dram_tensor("pat", [128, 4 * 64], bf16, kind="ExternalInput")
    # transposed output: [128 slot, wpc*128 (pair, feat)] -> host untransposes
    out = nc.dram_tensor("out", [128, own], bf16, kind="ExternalOutput")

    with tile.TileContext(nc) as tc, \
         tc.tile_pool(name="const", bufs=1) as constp, \
         tc.tile_pool(name="xgw", bufs=4) as xgwp, \
         tc.tile_pool(name="bfw", bufs=4) as bfwp, \
         tc.tile_pool(name="bps", bufs=3, space="PSUM") as bpsp, \
         tc.tile_pool(name="agg", bufs=2, space="PSUM") as aggp, \
         tc.tile_pool(name="bsb", bufs=3) as bsbp, \
         tc.tile_pool(name="comb", bufs=3) as combp, \
         tc.tile_pool(name="osb", bufs=2) as osbp:

        w3sb = constp.tile([128, 128], bf16)
        nc.sync.dma_start(w3sb[:], w3.ap())
        patsb = constp.tile([128, 4 * 64], bf16)

        # ramped group boundaries (all multiples of KPW) so the first
        # matmul starts after ~0.4MB instead of the full steady-state group
        gb = [0]
        for sz in (8, 16, 32):
            if gb[-1] + sz < NT:
                gb.append(gb[-1] + sz)
        while gb[-1] + TGX < NT:
            gb.append(gb[-1] + TGX)
        gb.append(NT)
        n_groups = len(gb) - 1
        gid = np.zeros(NT, np.int64)
        for i in range(n_groups):
            gid[gb[i]:gb[i + 1]] = i

        xg_groups = {}
        bf_groups = {}

        def load_group(gi):
            if gi < n_groups and gi not in xg_groups:
                lo, hi = gb[gi], gb[gi + 1]
                sz = hi - lo
                bt = bfwp.tile([128, (TGX // 2) * 128], bf16, tag="bfw")
                nc.sync.dma_start(bt[:, :(sz // 2) * 128],
                                  bfT.ap()[:, (lo // 2) * 128:(hi // 2) * 128])
                bf_groups[gi] = bt
                xt = xgwp.tile([128, TGX * 128], bf16, tag="xgw")
                nc.sync.dma_start(xt[:, :sz * 128],
                                  xgT.ap()[:, lo * 128:hi * 128])
                xg_groups[gi] = xt

        def group_tiles(T):
            gi = int(gid[T])
            load_group(gi)
            return xg_groups[gi], bf_groups[gi]

        def bonds_window(w):
            """bonds matmuls (row-tiled pairs) + evac + gate multiply -> cb.

            Tile j (A side, bank 1 cols j*128) runs in PE rows 0-63 while
            tile 4+j (B side, bank 2 cols 512+j*128) runs in rows 64-127."""
            bp = bpsp.tile([128, KPW * 128], f32, tag="bps")   # 2 psum banks
            T0 = w * KPW
            gi = int(gid[T0])
            xt, bt = group_tiles(T0)
            ko = ((T0 - gb[gi]) // 2) * 128         # bf col offset of pair w
            for j in range(4):
                nc.tensor.matmul(bp[:, j * 128:(j + 1) * 128],
                                 lhsT=bt[0:64, ko + j * 128:ko + (j + 1) * 128],
                                 rhs=w3sb[0:64, :],
                                 start=True, stop=True,
                                 tile_position=(0, 0))
                nc.tensor.matmul(bp[:, 512 + j * 128:512 + (j + 1) * 128],
                                 lhsT=bt[64:128, ko + j * 128:ko + (j + 1) * 128],
                                 rhs=w3sb[64:128, :],
                                 start=True, stop=True,
                                 tile_position=(64, 0))
            xt0 = xt
            xo = (T0 - gb[gi]) * 128
            cb = combp.tile([128, KPW * 128], bf16, tag="comb")
            if w % FUSE_MOD == FUSE_MOD - 1:
                # DVE fused: psum * sbuf -> sbuf (one 1024-wide op)
                nc.vector.tensor_tensor(out=cb[:], in0=bp[:],
                                        in1=xt0[:, xo:xo + KPW * 128],
                                        op=mybir.AluOpType.mult)
            else:
                # ACT evacuates psum, DVE multiplies bf16 at 2x
                bs = bsbp.tile([128, KPW * 128], bf16, tag="bsb")
                nc.scalar.copy(bs[:], bp[:])
                nc.vector.tensor_tensor(out=cb[:], in0=bs[:],
                                        in1=xt0[:, xo:xo + KPW * 128],
                                        op=mybir.AluOpType.mult)
            return cb

        cbq = [bonds_window(0)]
        nc.sync.dma_start(patsb[:], pat.ap())
        ob = None
        agg = None
        for w in range(wpc):
            # prefetch xg/bf groups two ahead of consumption
            gi_now = int(gid[min(w * KPW, NT - 1)])
            load_group(gi_now + 1)
            load_group(gi_now + 2)
            if w % 4 == 0:
                agg = aggp.tile([128, 512], f32, tag="agg")
            wb = w % 4
            if w % OG == 0:
                ob = osbp.tile([128, OG * 128], bf16, tag="osb")

            cb = cbq.pop(0)
            if w + 1 < wpc:
                # software pipeline: bonds one pair ahead of scatter(w)
                cbq.append(bonds_window(w + 1))

            # scatter: col-tiled pairs of fixed-pattern matmuls; side A
            # (slots 0-63) and side B (slots 64-127) run concurrently in
            # separate column groups of the PE array
            for t in range(4):
                nc.tensor.matmul(agg[0:64, wb * 128:(wb + 1) * 128],
                                 lhsT=patsb[:, t * 64:(t + 1) * 64],
                                 rhs=cb[:, t * 128:(t + 1) * 128],
                                 start=(t == 0), stop=(t == 3),
                                 tile_position=(0, 0))
                nc.tensor.matmul(agg[64:128, wb * 128:(wb + 1) * 128],
                                 lhsT=patsb[:, t * 64:(t + 1) * 64],
                                 rhs=cb[:, 512 + t * 128:512 + (t + 1) * 128],
                                 start=(t == 0), stop=(t == 3),
                                 tile_position=(0, 64))

            # output add for a finished agg bank
            if wb == 3 or w == wpc - 1:
                nb = wb + 1                      # windows in this bank
                w0 = w - wb                      # first window of bank
                j0 = w0 % OG
                nc.vector.tensor_scalar(
                    ob[:, j0 * 128:(j0 + nb) * 128],
                    agg[:, :nb * 128], 1.0, None,
                    mybir.AluOpType.mult)
            if (w % OG == OG - 1 or w == wpc - 1):
                j = w % OG
                w0 = w - j
                nc.sync.dma_start(out.ap()[:, w0 * 128:(w0 + j + 1) * 128],
                                  ob[:, :(j + 1) * 128])

    nc.compile()
    _prog_cache[key] = nc
    return nc


def _fold_bn(W, b, gamma, beta, mean, var):
    s = (gamma.astype(np.float64) / np.sqrt(var.astype(np.float64) + BN_EPS))
    Wp = W.astype(np.float64) * s[None, :]
    c = (b.astype(np.float64) - mean.astype(np.float64)) * s \
        + beta.astype(np.float64)
    return Wp, c


NS = 64            # slots per (half-)window
CAPTOT = 512       # edge capacity per window (= 4 tiles of 128)


def _make_schedule(deg, n_atoms):
    """Assign atoms (split into chunks of degree <= capmax) to
    (window, slot) so that every 64-slot window has the same
    slot-capacity profile summing to exactly CAPTOT."""
    best = None
    for wpc in (98, 99, 100, 101, 102, 104, 106):
        W = 2 * NCORES * wpc                 # 64-slot windows total
        for capmax in (15, 14, 13):
            nch = np.maximum(1, -(-deg // capmax))       # chunks per atom
            C = int(nch.sum())
            if C > W * NS:
                continue
            reps = nch
            base = deg // reps
            rem = deg - base * reps
            atom_of_chunk = np.repeat(np.arange(n_atoms), reps)
            idx_in_atom = np.arange(C) - np.repeat(
                np.cumsum(reps) - reps, reps)
            cdeg = (np.repeat(base, reps)
                    + (idx_in_atom < np.repeat(rem, reps))).astype(np.int64)
            order = np.argsort(-cdeg, kind="stable")
            cs = cdeg[order]
            nblk = -(-C // W)
            if nblk > NS:
                continue
            prof = np.zeros(NS, np.int64)
            prof[:nblk] = cs[np.arange(nblk) * W]
            S = int(prof.sum())
            if S > CAPTOT:
                continue
            add = CAPTOT - S
            k = 0
            while add > 0:
                prof[k % NS] += 1
                add -= 1
                k += 1
            prof = np.sort(prof)[::-1].copy()
            tiles = NCORES * wpc * KPW
            cand = (tiles, wpc, capmax, prof, order, cs,
                    atom_of_chunk, idx_in_atom, reps)
            if best is None or cand[0] < best[0]:
                best = cand
        if best is not None:
            break
    assert best is not None, "no feasible schedule"
    return best


def _prepare(inputs):
    X = np.asarray(inputs["atom_features"], np.float32)
    BF = np.asarray(inputs["bond_features"], np.float32)
    BP = np.asarray(inputs["bond_pairs"], np.int32)
    n_atoms = X.shape[0]

    W1p, c1 = _fold_bn(np.asarray(inputs["W1"]), np.asarray(inputs["b1"]),
                       np.asarray(inputs["g1"]), np.asarray(inputs["be1"]),
                       np.asarray(inputs["m1"]), np.asarray(inputs["v1"]))
    W2p, c2 = _fold_bn(np.asarray(inputs["W2"]), np.asarray(inputs["b2"]),
                       np.asarray(inputs["g2"]), np.asarray(inputs["be2"]),
                       np.asarray(inputs["m2"]), np.asarray(inputs["v2"]))
    W3p, c3 = _fold_bn(np.asarray(inputs["W3"]), np.asarray(inputs["b3"]),
                       np.asarray(inputs["g3"]), np.asarray(inputs["be3"]),
                       np.asarray(inputs["m3"]), np.asarray(inputs["v3"]))
    W12 = W1p @ W2p
    c12 = c1 @ W2p + c2

    X12 = (X.astype(np.float64) @ W12).astype(np.float32)   # [N, 128]

    dest = BP[:, 0].astype(np.int64)
    src = BP[:, 1].astype(np.int64)

    # sort edges by dest
    perm = np.argsort(dest, kind="stable")
    ds, ss = dest[perm], src[perm]
    bfs = BF[perm]

    deg = np.bincount(ds, minlength=n_atoms).astype(np.int64)

    # host-folded bias terms (incl. atom_h = X@W1p + c1):
    uniq, idxstart = np.unique(ds, return_index=True)
    part_bf = np.add.reduceat(bfs.astype(np.float64), idxstart, axis=0)
    sbsum = np.zeros((n_atoms, BF.shape[1]))
    sbsum[uniq] = part_bf
    part_x = np.add.reduceat(X12[ss].astype(np.float64), idxstart, axis=0)
    sx12 = np.zeros((n_atoms, 128))
    sx12[uniq] = part_x
    Zh = ((sbsum @ W3p) * c12[None, :]
          + deg.astype(np.float64)[:, None] * (c3 * c12)[None, :]
          + sx12 * c3[None, :]
          + X.astype(np.float64) @ W1p + c1[None, :]).astype(np.float32)

    (tiles, wpc, capmax, prof, order, cs, atom_of_chunk,
     idx_in_atom, reps) = _make_schedule(deg, n_atoms)
    W = 2 * NCORES * wpc                    # 64-slot windows
    NT = wpc * KPW
    own = wpc * 128

    # chunk rank r (desc order) -> window r % W, slot position r // W
    C = len(order)
    win_of_chunk = np.empty(C, np.int64)
    slot_of_chunk = np.empty(C, np.int64)
    win_of_chunk[order] = np.arange(C) % W
    slot_of_chunk[order] = np.arange(C) // W

    prof_prefix = np.zeros(NS + 1, np.int64)
    prof_prefix[1:] = np.cumsum(prof)

    # window -> (core, pair, side)
    core_of_win = win_of_chunk // (2 * wpc)
    wloc2 = win_of_chunk - core_of_win * (2 * wpc)
    pair_of_win = wloc2 // 2
    side_of_win = wloc2 % 2
    # instance index within the 64-slot window (0..511)
    inst_in_win = prof_prefix[slot_of_chunk]

    # edge -> chunk mapping: edges of atom a sorted; chunk boundaries at
    # offsets (cumsum of cdeg within atom)
    # chunk edge start (within dest-sorted edge array):
    atom_run_start = np.zeros(n_atoms, np.int64)
    atom_run_start[1:] = np.cumsum(deg)[:-1]
    # cdeg in chunk-id order (cs is rank order)
    cdeg_chunkid = np.empty(C, np.int64)
    cdeg_chunkid[order] = cs
    # offset of chunk within its atom = cumsum of previous chunk degrees
    # chunks of an atom are consecutive chunk ids; use segmented cumsum
    seg_start = np.cumsum(reps) - reps          # first chunk id per atom
    csum = np.cumsum(cdeg_chunkid)
    prev = np.zeros(C, np.int64)
    prev[1:] = csum[:-1]
    atom_first_prev = prev[seg_start]           # cumsum before atom's chunks
    chunk_off_in_atom = prev - np.repeat(atom_first_prev, reps)
    chunk_edge_start = np.repeat(atom_run_start, reps) + chunk_off_in_atom

    # per-edge instance position within window, then global stream pos:
    # tile t of pair p is side A for even t, side B for odd t
    epos = np.empty(len(ds), np.int64)
    nz = cdeg_chunkid > 0
    starts = chunk_edge_start[nz]
    lens = cdeg_chunkid[nz]
    tot = int(lens.sum())
    assert tot == len(ds)
    seg_off = np.repeat(np.cumsum(lens) - lens, lens)
    within = np.arange(tot) - seg_off
    edge_idx = np.repeat(starts, lens) + within
    ii = np.repeat(inst_in_win[nz], lens) + within      # 0..511 in window
    e_core = np.repeat(core_of_win[nz], lens)
    e_pair = np.repeat(pair_of_win[nz], lens)
    e_side = np.repeat(side_of_win[nz], lens)
    e_tile = ii // 128
    e_row = ii - e_tile * 128
    epos[edge_idx] = ((e_core * wpc + e_pair) * KPW
                      + e_side * 4 + e_tile) * 128 + e_row

    TOT = NCORES * NT * 128
    X12b = X12.astype(BF16)
    xgE = np.zeros((TOT, 128), BF16)
    xgE[epos] = X12b[ss]
    bfE = np.zeros((TOT, F_BOND), BF16)
    bfE[epos] = bfs.astype(BF16)

    prim = idx_in_atom == 0
    pos_row = pair_of_win * 128 + side_of_win * 64 + slot_of_chunk

    # fixed patterns: slot id per instance (4 tiles of 128 per window)
    slot_of_inst = np.repeat(np.arange(NS), prof)
    patm = np.zeros((4, 128, NS), np.float32)
    for t in range(4):
        patm[t, np.arange(128), slot_of_inst[t * 128:(t + 1) * 128]] = 1
    pat = np.ascontiguousarray(
        patm.transpose(1, 0, 2).reshape(128, 4 * NS).astype(BF16))

    w3b = np.concatenate([W3p, W3p], axis=0)          # [128, 128]
    consts = dict(w3=np.ascontiguousarray(w3b.astype(BF16)), pat=pat)

    in_maps = []
    for c in range(NCORES):
        sl = slice(c * NT * 128, (c + 1) * NT * 128)
        m = dict(consts)
        m["xgT"] = np.ascontiguousarray(
            xgE[sl].reshape(NT, 128, 128).transpose(1, 0, 2)
            .reshape(128, NT * 128))
        # pack bf features: partitions 0-63 = A tiles (0-3 of each pair),
        # 64-127 = B tiles (4-7), col block j of pair p = bf col p*4+j
        bfc = bfE[sl].reshape(NT // KPW, KPW, 128, F_BOND)
        blocks = np.concatenate(
            [bfc[:, 0:4].transpose(0, 1, 3, 2),
             bfc[:, 4:8].transpose(0, 1, 3, 2)], axis=2)   # [p, 4, 128, 128]
        m["bfT"] = np.ascontiguousarray(
            blocks.transpose(2, 0, 1, 3).reshape(128, (NT // 2) * 128))
        in_maps.append(m)

    # output merge info
    merge = dict(core=core_of_win, row=pos_row, atom=atom_of_chunk,
                 prim=prim, wpc=wpc, prof=tuple(int(x) for x in prof),
                 Zh=Zh)
    return in_maps, merge


def run(inputs):
    global LAST_RESULTS
    in_maps, merge = _prepare(inputs)
    wpc = merge["wpc"]
    nc = _build_program(wpc, merge["prof"])
    res = run_bass_kernel_spmd(nc, in_maps, core_ids=list(range(NCORES)),
                               trace=TRACE)
    LAST_RESULTS = res
    own = wpc * 128
    od = np.stack([res.results[c]["out"].astype(np.float32)
                   .reshape(128, wpc, 128).transpose(1, 0, 2)
                   .reshape(own, 128)
                   for c in range(NCORES)])        # [8, own, 128]
    n_atoms = N
    out = np.zeros((n_atoms, 128), np.float32)
    core, row, atom, prim = (merge["core"], merge["row"], merge["atom"],
                             merge["prim"])
    out[atom[prim]] = od[core[prim], row[prim]]
    sec = ~prim
    if sec.any():
        np.add.at(out, atom[sec], od[core[sec], row[sec]])
    out += merge["Zh"]
    return out


def kernel(**inputs):
    return run(inputs)


# revision 5
# speedup vs baseline: 1.0223x; 1.0223x over previous
"""Trainium2 Bass kernel for GraphConvolution message passing.

Computation (reference):
    atom_h = BN1(X @ W1)                       # [N, 128]
    neigh  = BN2(atom_h[src] @ W2)             # [E, 128]
    bonds  = BN3(bond_features @ W3)           # [E, 128]
    agg    = segment_sum(neigh * bonds, dest)  # [N, 128]
    out    = atom_h + agg

Strategy: fold BN + W1@W2 on the host, pre-gather neighbor features per
edge (xg = (X@W12)[src]) and stream them.  Per core the device runs a
3-stage pipeline per 1024-edge window pair:
  1. bonds matmuls as ROW-TILED K=64 pairs (two tiles concurrently in PE
     rows 0-63 / 64-127, W3 duplicated in both halves),
  2. PSUM evacuation (ACT) + gate multiply (DVE, bf16 2x),
  3. scatter as COL-TILED M=64 pairs of FIXED one-hot pattern matmuls
     accumulating into PSUM.
The fixed scatter patterns are possible because the host assigns atoms to
window slots so every 64-slot window has the same degree-capacity profile
(high-degree atoms are split into chunks and re-merged on the host); this
removes all per-tile one-hot construction and index streams.
Zh (all bias/BN terms + atom_h) is added on the host after download.
"""

import numpy as np
import ml_dtypes

import concourse.bass as bass
import concourse.tile as tile
from concourse import bacc, mybir
from concourse.bass_utils import run_bass_kernel_spmd

BF16 = ml_dtypes.bfloat16
BN_EPS = 1e-3

N, E, F_ATOM, F_BOND, U = 100000, 800000, 128, 64, 128
NCORES = 8
KPW = 8            # tiles (of 128 edge slots) per window: capacity 1024

TRACE = False
LAST_RESULTS = None

_prog_cache = {}

TGX = 64           # tiles per xg/bf DMA group
OG = 16            # window-pairs per output DMA
FUSE_MOD = 1000    # effectively off: ACT evacuates every pair


def _build_program(wpc, profile):
    """profile: tuple of 128 slot capacities (desc), sum == 1024."""
    key = (wpc, profile)
    if key in _prog_cache:
        return _prog_cache[key]

    NT = wpc * KPW
    own = wpc * 128
    f32, bf16 = mybir.dt.float32, mybir.dt.bfloat16

    nc = bacc.Bacc("TRN2", target_bir_lowering=False, debug=False,
                   num_devices=NCORES)

    xgT = nc.dram_tensor("xgT", [128, NT * 128], bf16, kind="ExternalInput")
    # bf features packed in partition halves: rows 0-63 = tiles 0-3 of each
    # pair (A side), rows 64-127 = tiles 4-7 (B side)
    bfT = nc.dram_tensor("bfT", [128, (NT // 2) * 128], bf16,
                         kind="ExternalInput")
    # W3 duplicated into both partition halves
    w3 = nc.dram_tensor("w3", [128, 128], bf16, kind="ExternalInput")
    # 4 fixed one-hot patterns of [128 edge, 64 slot] (shared by A/B sides)
    pat = nc.dram_tensor("pat", [128, 4 * 64], bf16, kind="ExternalInput")
    # transposed output: [128 slot, wpc*128 (pair, feat)] -> host untransposes
    out = nc.dram_tensor("out", [128, own], bf16, kind="ExternalOutput")

    with tile.TileContext(nc) as tc, \
         tc.tile_pool(name="const", bufs=1) as constp, \
         tc.tile_pool(name="xgw", bufs=5) as xgwp, \
         tc.tile_pool(name="bfw", bufs=5) as bfwp, \
         tc.tile_pool(name="bps", bufs=3, space="PSUM") as bpsp, \
         tc.tile_pool(name="agg", bufs=2, space="PSUM") as aggp, \
         tc.tile_pool(name="bsb", bufs=3) as bsbp, \
         tc.tile_pool(name="comb", bufs=3) as combp, \
         tc.tile_pool(name="osb", bufs=2) as osbp:

        w3sb = constp.tile([128, 128], bf16)
        nc.sync.dma_start(w3sb[:], w3.ap())
        patsb = constp.tile([128, 4 * 64], bf16)

        # ramped group boundaries (all multiples of KPW) so the first
        # matmul starts after ~0.4MB instead of the full steady-state group
        gb = [0]
        for sz in (8, 16, 32):
            if gb[-1] + sz < NT:
                gb.append(gb[-1] + sz)
        while gb[-1] + TGX < NT:
            gb.append(gb[-1] + TGX)
        gb.append(NT)
        n_groups = len(gb) - 1
        gid = np.zeros(NT, np.int64)
        for i in range(n_groups):
            gid[gb[i]:gb[i + 1]] = i

        xg_groups = {}
        bf_groups = {}

        def load_group(gi):
            if gi < n_groups and gi not in xg_groups:
                lo, hi = gb[gi], gb[gi + 1]
                sz = hi - lo
                bt = bfwp.tile([128, (TGX // 2) * 128], bf16, tag="bfw")
                nc.sync.dma_start(bt[:, :(sz // 2) * 128],
                                  bfT.ap()[:, (lo // 2) * 128:(hi // 2) * 128])
                bf_groups[gi] = bt
                xt = xgwp.tile([128, TGX * 128], bf16, tag="xgw")
                nc.sync.dma_start(xt[:, :sz * 128],
                                  xgT.ap()[:, lo * 128:hi * 128])
                xg_groups[gi] = xt

        def group_tiles(T):
            gi = int(gid[T])
            load_group(gi)
            return xg_groups[gi], bf_groups[gi]

        def bonds_window(w):
            """bonds matmuls (row-tiled pairs) + evac + gate multiply -> cb.

            Tile j (A side, bank 1 cols j*128) runs in PE rows 0-63 while
            tile 4+j (B side, bank 2 cols 512+j*128) runs in rows 64-127."""
            bp = bpsp.tile([128, KPW * 128], f32, tag="bps")   # 2 psum banks
            T0 = w * KPW
            gi = int(gid[T0])
            xt, bt = group_tiles(T0)
            ko = ((T0 - gb[gi]) // 2) * 128         # bf col offset of pair w
            for j in range(4):
                nc.tensor.matmul(bp[:, j * 128:(j + 1) * 128],
                                 lhsT=bt[0:64, ko + j * 128:ko + (j + 1) * 128],
                                 rhs=w3sb[0:64, :],
                                 start=True, stop=True,
                                 tile_position=(0, 0))
                nc.tensor.matmul(bp[:, 512 + j * 128:512 + (j + 1) * 128],
                                 lhsT=bt[64:128, ko + j * 128:ko + (j + 1) * 128],
                                 rhs=w3sb[64:128, :],
                                 start=True, stop=True,
                                 tile_position=(64, 0))
            xt0 = xt
            xo = (T0 - gb[gi]) * 128
            cb = combp.tile([128, KPW * 128], bf16, tag="comb")
            if w % FUSE_MOD == FUSE_MOD - 1:
                # DVE fused: psum * sbuf -> sbuf (one 1024-wide op)
                nc.vector.tensor_tensor(out=cb[:], in0=bp[:],
                                        in1=xt0[:, xo:xo + KPW * 128],
                                        op=mybir.AluOpType.mult)
            else:
                # ACT evacuates psum, DVE multiplies bf16 at 2x
                bs = bsbp.tile([128, KPW * 128], bf16, tag="bsb")
                nc.scalar.copy(bs[:], bp[:])
                nc.vector.tensor_tensor(out=cb[:], in0=bs[:],
                                        in1=xt0[:, xo:xo + KPW * 128],
                                        op=mybir.AluOpType.mult)
            return cb

        cbq = [bonds_window(0)]
        nc.sync.dma_start(patsb[:], pat.ap())
        ob = None
        agg = None
        for w in range(wpc):
            # prefetch xg/bf groups two ahead of consumption
            gi_now = int(gid[min(w * KPW, NT - 1)])
            load_group(gi_now + 1)
            load_group(gi_now + 2)
            if w % 4 == 0:
                agg = aggp.tile([128, 512], f32, tag="agg")
            wb = w % 4
            if w % OG == 0:
                ob = osbp.tile([128, OG * 128], bf16, tag="osb")

            cb = cbq.pop(0)
            if w + 1 < wpc:
                # software pipeline: bonds one pair ahead of scatter(w)
                cbq.append(bonds_window(w + 1))

            # scatter: col-tiled pairs of fixed-pattern matmuls; side A
            # (slots 0-63) and side B (slots 64-127) run concurrently in
            # separate column groups of the PE array
            for t in range(4):
                nc.tensor.matmul(agg[0:64, wb * 128:(wb + 1) * 128],
                                 lhsT=patsb[:, t * 64:(t + 1) * 64],
                                 rhs=cb[:, t * 128:(t + 1) * 128],
                                 start=(t == 0), stop=(t == 3),
                                 tile_position=(0, 0))
                nc.tensor.matmul(agg[64:128, wb * 128:(wb + 1) * 128],
                                 lhsT=patsb[:, t * 64:(t + 1) * 64],
                                 rhs=cb[:, 512 + t * 128:512 + (t + 1) * 128],
                                 start=(t == 0), stop=(t == 3),
                                 tile_position=(0, 64))

            # output add for a finished agg bank
            if wb == 3 or w == wpc - 1:
                nb = wb + 1                      # windows in this bank
                w0 = w - wb                      # first window of bank
                j0 = w0 % OG
                nc.vector.tensor_scalar(
                    ob[:, j0 * 128:(j0 + nb) * 128],
                    agg[:, :nb * 128], 1.0, None,
                    mybir.AluOpType.mult)
            if (w % OG == OG - 1 or w == wpc - 1):
                j = w % OG
                w0 = w - j
                nc.sync.dma_start(out.ap()[:, w0 * 128:(w0 + j + 1) * 128],
                                  ob[:, :(j + 1) * 128])

    nc.compile()
    _prog_cache[key] = nc
    return nc


def _fold_bn(W, b, gamma, beta, mean, var):
    s = (gamma.astype(np.float64) / np.sqrt(var.astype(np.float64) + BN_EPS))
    Wp = W.astype(np.float64) * s[None, :]
    c = (b.astype(np.float64) - mean.astype(np.float64)) * s \
        + beta.astype(np.float64)
    return Wp, c


NS = 64            # slots per (half-)window
CAPTOT = 512       # edge capacity per window (= 4 tiles of 128)


def _make_schedule(deg, n_atoms):
    """Assign atoms (split into chunks of degree <= capmax) to
    (window, slot) so that every 64-slot window has the same
    slot-capacity profile summing to exactly CAPTOT."""
    best = None
    for wpc in (98, 99, 100, 101, 102, 104, 106):
        W = 2 * NCORES * wpc                 # 64-slot windows total
        for capmax in (15, 14, 13):
            nch = np.maximum(1, -(-deg // capmax))       # chunks per atom
            C = int(nch.sum())
            if C > W * NS:
                continue
            reps = nch
            base = deg // reps
            rem = deg - base * reps
            atom_of_chunk = np.repeat(np.arange(n_atoms), reps)
            idx_in_atom = np.arange(C) - np.repeat(
                np.cumsum(reps) - reps, reps)
            cdeg = (np.repeat(base, reps)
                    + (idx_in_atom < np.repeat(rem, reps))).astype(np.int64)
            order = np.argsort(-cdeg, kind="stable")
            cs = cdeg[order]
            nblk = -(-C // W)
            if nblk > NS:
                continue
            prof = np.zeros(NS, np.int64)
            prof[:nblk] = cs[np.arange(nblk) * W]
            S = int(prof.sum())
            if S > CAPTOT:
                continue
            add = CAPTOT - S
            k = 0
            while add > 0:
                prof[k % NS] += 1
                add -= 1
                k += 1
            prof = np.sort(prof)[::-1].copy()
            tiles = NCORES * wpc * KPW
            cand = (tiles, wpc, capmax, prof, order, cs,
                    atom_of_chunk, idx_in_atom, reps)
            if best is None or cand[0] < best[0]:
                best = cand
        if best is not None:
            break
    assert best is not None, "no feasible schedule"
    return best


def _prepare(inputs):
    X = np.asarray(inputs["atom_features"], np.float32)
    BF = np.asarray(inputs["bond_features"], np.float32)
    BP = np.asarray(inputs["bond_pairs"], np.int32)
    n_atoms = X.shape[0]

    W1p, c1 = _fold_bn(np.asarray(inputs["W1"]), np.asarray(inputs["b1"]),
                       np.asarray(inputs["g1"]), np.asarray(inputs["be1"]),
                       np.asarray(inputs["m1"]), np.asarray(inputs["v1"]))
    W2p, c2 = _fold_bn(np.asarray(inputs["W2"]), np.asarray(inputs["b2"]),
                       np.asarray(inputs["g2"]), np.asarray(inputs["be2"]),
                       np.asarray(inputs["m2"]), np.asarray(inputs["v2"]))
    W3p, c3 = _fold_bn(np.asarray(inputs["W3"]), np.asarray(inputs["b3"]),
                       np.asarray(inputs["g3"]), np.asarray(inputs["be3"]),
                       np.asarray(inputs["m3"]), np.asarray(inputs["v3"]))
    W12 = W1p @ W2p
    c12 = c1 @ W2p + c2

    X12 = (X.astype(np.float64) @ W12).astype(np.float32)   # [N, 128]

    dest = BP[:, 0].astype(np.int64)
    src = BP[:, 1].astype(np.int64)

    # sort edges by dest
    perm = np.argsort(dest, kind="stable")
    ds, ss = dest[perm], src[perm]
    bfs = BF[perm]

    deg = np.bincount(ds, minlength=n_atoms).astype(np.int64)

    # host-folded bias terms (incl. atom_h = X@W1p + c1):
    uniq, idxstart = np.unique(ds, return_index=True)
    part_bf = np.add.reduceat(bfs.astype(np.float64), idxstart, axis=0)
    sbsum = np.zeros((n_atoms, BF.shape[1]))
    sbsum[uniq] = part_bf
    part_x = np.add.reduceat(X12[ss].astype(np.float64), idxstart, axis=0)
    sx12 = np.zeros((n_atoms, 128))
    sx12[uniq] = part_x
    Zh = ((sbsum @ W3p) * c12[None, :]
          + deg.astype(np.float64)[:, None] * (c3 * c12)[None, :]
          + sx12 * c3[None, :]
          + X.astype(np.float64) @ W1p + c1[None, :]).astype(np.float32)

    (tiles, wpc, capmax, prof, order, cs, atom_of_chunk,
     idx_in_atom, reps) = _make_schedule(deg, n_atoms)
    W = 2 * NCORES * wpc                    # 64-slot windows
    NT = wpc * KPW
    own = wpc * 128

    # chunk rank r (desc order) -> window r % W, slot position r // W
    C = len(order)
    win_of_chunk = np.empty(C, np.int64)
    slot_of_chunk = np.empty(C, np.int64)
    win_of_chunk[order] = np.arange(C) % W
    slot_of_chunk[order] = np.arange(C) // W

    prof_prefix = np.zeros(NS + 1, np.int64)
    prof_prefix[1:] = np.cumsum(prof)

    # window -> (core, pair, side)
    core_of_win = win_of_chunk // (2 * wpc)
    wloc2 = win_of_chunk - core_of_win * (2 * wpc)
    pair_of_win = wloc2 // 2
    side_of_win = wloc2 % 2
    # instance index within the 64-slot window (0..511)
    inst_in_win = prof_prefix[slot_of_chunk]

    # edge -> chunk mapping: edges of atom a sorted; chunk boundaries at
    # offsets (cumsum of cdeg within atom)
    # chunk edge start (within dest-sorted edge array):
    atom_run_start = np.zeros(n_atoms, np.int64)
    atom_run_start[1:] = np.cumsum(deg)[:-1]
    # cdeg in chunk-id order (cs is rank order)
    cdeg_chunkid = np.empty(C, np.int64)
    cdeg_chunkid[order] = cs
    # offset of chunk within its atom = cumsum of previous chunk degrees
    # chunks of an atom are consecutive chunk ids; use segmented cumsum
    seg_start = np.cumsum(reps) - reps          # first chunk id per atom
    csum = np.cumsum(cdeg_chunkid)
    prev = np.zeros(C, np.int64)
    prev[1:] = csum[:-1]
    atom_first_prev = prev[seg_start]           # cumsum before atom's chunks
    chunk_off_in_atom = prev - np.repeat(atom_first_prev, reps)
    chunk_edge_start = np.repeat(atom_run_start, reps) + chunk_off_in_atom

    # per-edge instance position within window, then global stream pos:
    # tile t of pair p is side A for even t, side B for odd t
    epos = np.empty(len(ds), np.int64)
    nz = cdeg_chunkid > 0
    starts = chunk_edge_start[nz]
    lens = cdeg_chunkid[nz]
    tot = int(lens.sum())
    assert tot == len(ds)
    seg_off = np.repeat(np.cumsum(lens) - lens, lens)
    within = np.arange(tot) - seg_off
    edge_idx = np.repeat(starts, lens) + within
    ii = np.repeat(inst_in_win[nz], lens) + within      # 0..511 in window
    e_core = np.repeat(core_of_win[nz], lens)
    e_pair = np.repeat(pair_of_win[nz], lens)
    e_side = np.repeat(side_of_win[nz], lens)
    e_tile = ii // 128
    e_row = ii - e_tile * 128
    epos[edge_idx] = ((e_core * wpc + e_pair) * KPW
                      + e_side * 4 + e_tile) * 128 + e_row

    TOT = NCORES * NT * 128
    X12b = X12.astype(BF16)
    xgE = np.zeros((TOT, 128), BF16)
    xgE[epos] = X12b[ss]
    bfE = np.zeros((TOT, F_BOND), BF16)
    bfE[epos] = bfs.astype(BF16)

    prim = idx_in_atom == 0
    pos_row = pair_of_win * 128 + side_of_win * 64 + slot_of_chunk

    # fixed patterns: slot id per instance (4 tiles of 128 per window)
    slot_of_inst = np.repeat(np.arange(NS), prof)
    patm = np.zeros((4, 128, NS), np.float32)
    for t in range(4):
        patm[t, np.arange(128), slot_of_inst[t * 128:(t + 1) * 128]] = 1
    pat = np.ascontiguousarray(
        patm.transpose(1, 0, 2).reshape(128, 4 * NS).astype(BF16))

    w3b = np.concatenate([W3p, W3p], axis=0)          # [128, 128]
    consts = dict(w3=np.ascontiguousarray(w3b.astype(BF16)), pat=pat)

    in_maps = []
    for c in range(NCORES):
        sl = slice(c * NT * 128, (c + 1) * NT * 128)
        m = dict(consts)
        m["xgT"] = np.ascontiguousarray(
            xgE[sl].reshape(NT, 128, 128).transpose(1, 0, 2)
            .reshape(128, NT * 128))
        # pack bf features: partitions 0-63 = A tiles (0-3 of each pair),
        # 64-127 = B tiles (4-7), col block j of pair p = bf col p*4+j
        bfc = bfE[sl].reshape(NT // KPW, KPW, 128, F_BOND)
        blocks = np.concatenate(
            [bfc[:, 0:4].transpose(0, 1, 3, 2),
             bfc[:, 4:8].transpose(0, 1, 3, 2)], axis=2)   # [p, 4, 128, 128]
        m["bfT"] = np.ascontiguousarray(
            blocks.transpose(2, 0, 1, 3).reshape(128, (NT // 2) * 128))
        in_maps.append(m)

    # output merge info
    merge = dict(core=core_of_win, row=pos_row, atom=atom_of_chunk,
                 prim=prim, wpc=wpc, prof=tuple(int(x) for x in prof),
                 Zh=Zh)
    return in_maps, merge


def run(inputs):
    global LAST_RESULTS
    in_maps, merge = _prepare(inputs)
    wpc = merge["wpc"]
    nc = _build_program(wpc, merge["prof"])
    res = run_bass_kernel_spmd(nc, in_maps, core_ids=list(range(NCORES)),
                               trace=TRACE)
    LAST_RESULTS = res
    own = wpc * 128
    od = np.stack([res.results[c]["out"].astype(np.float32)
                   .reshape(128, wpc, 128).transpose(1, 0, 2)
                   .reshape(own, 128)
                   for c in range(NCORES)])        # [8, own, 128]
    n_atoms = N
    out = np.zeros((n_atoms, 128), np.float32)
    core, row, atom, prim = (merge["core"], merge["row"], merge["atom"],
                             merge["prim"])
    out[atom[prim]] = od[core[prim], row[prim]]
    sec = ~prim
    if sec.any():
        np.add.at(out, atom[sec], od[core[sec], row[sec]])
    out += merge["Zh"]
    return out


def kernel(**inputs):
    return run(inputs)


# revision 6
# speedup vs baseline: 1.0331x; 1.0106x over previous
"""Trainium2 Bass kernel for GraphConvolution message passing.

Computation (reference):
    atom_h = BN1(X @ W1)                       # [N, 128]
    neigh  = BN2(atom_h[src] @ W2)             # [E, 128]
    bonds  = BN3(bond_features @ W3)           # [E, 128]
    agg    = segment_sum(neigh * bonds, dest)  # [N, 128]
    out    = atom_h + agg

Strategy: fold BN + W1@W2 on the host, pre-gather neighbor features per
edge (xg = (X@W12)[src]) and stream them.  Per core the device runs a
3-stage pipeline per 1024-edge window pair:
  1. bonds matmuls as ROW-TILED K=64 pairs (two tiles concurrently in PE
     rows 0-63 / 64-127, W3 duplicated in both halves),
  2. PSUM evacuation (ACT) + gate multiply (DVE, bf16 2x),
  3. scatter as COL-TILED M=64 pairs of FIXED one-hot pattern matmuls
     accumulating into PSUM.
The fixed scatter patterns are possible because the host assigns atoms to
window slots so every 64-slot window has the same degree-capacity profile
(high-degree atoms are split into chunks and re-merged on the host); this
removes all per-tile one-hot construction and index streams.
Zh (all bias/BN terms + atom_h) is added on the host after download.
"""

import numpy as np
import ml_dtypes

import concourse.bass as bass
import concourse.tile as tile
from concourse import bacc, mybir
from concourse.bass_utils import run_bass_kernel_spmd

BF16 = ml_dtypes.bfloat16
BN_EPS = 1e-3

N, E, F_ATOM, F_BOND, U = 100000, 800000, 128, 64, 128
NCORES = 8
KPW = 8            # tiles (of 128 edge slots) per window: capacity 1024

TRACE = False
LAST_RESULTS = None

_prog_cache = {}

TGX = 64           # tiles per xg/bf DMA group
OG = 16            # window-pairs per output DMA
FUSE_MOD = 1000    # effectively off: ACT evacuates every pair


def _build_program(wpc, profile):
    """profile: tuple of 128 slot capacities (desc), sum == 1024."""
    key = (wpc, profile)
    if key in _prog_cache:
        return _prog_cache[key]

    NT = wpc * KPW
    own = wpc * 128
    f32, bf16 = mybir.dt.float32, mybir.dt.bfloat16

    nc = bacc.Bacc("TRN2", target_bir_lowering=False, debug=False,
                   num_devices=NCORES)

    xgT = nc.dram_tensor("xgT", [128, NT * 128], bf16, kind="ExternalInput")
    # bf features packed in partition halves: rows 0-63 = tiles 0-3 of each
    # pair (A side), rows 64-127 = tiles 4-7 (B side)
    bfT = nc.dram_tensor("bfT", [128, (NT // 2) * 128], bf16,
                         kind="ExternalInput")
    # W3 duplicated into both partition halves
    w3 = nc.dram_tensor("w3", [128, 128], bf16, kind="ExternalInput")
    # 4 fixed one-hot patterns of [128 edge, 64 slot] (shared by A/B sides)
    pat = nc.dram_tensor("pat", [128, 4 * 64], bf16, kind="ExternalInput")
    # transposed output: [128 slot, wpc*128 (pair, feat)] -> host untransposes
    out = nc.dram_tensor("out", [128, own], bf16, kind="ExternalOutput")

    with tile.TileContext(nc) as tc, \
         tc.tile_pool(name="const", bufs=1) as constp, \
         tc.tile_pool(name="xgw", bufs=6) as xgwp, \
         tc.tile_pool(name="bfw", bufs=6) as bfwp, \
         tc.tile_pool(name="bps", bufs=3, space="PSUM") as bpsp, \
         tc.tile_pool(name="agg", bufs=2, space="PSUM") as aggp, \
         tc.tile_pool(name="bsb", bufs=3) as bsbp, \
         tc.tile_pool(name="comb", bufs=3) as combp, \
         tc.tile_pool(name="osb", bufs=2) as osbp:

        w3sb = constp.tile([128, 128], bf16)
        nc.sync.dma_start(w3sb[:], w3.ap())
        patsb = constp.tile([128, 4 * 64], bf16)

        # ramped group boundaries (all multiples of KPW) so the first
        # matmul starts after ~0.4MB instead of the full steady-state group
        gb = [0]
        for sz in (8, 16, 32):
            if gb[-1] + sz < NT:
                gb.append(gb[-1] + sz)
        while gb[-1] + TGX < NT:
            gb.append(gb[-1] + TGX)
        gb.append(NT)
        n_groups = len(gb) - 1
        gid = np.zeros(NT, np.int64)
        for i in range(n_groups):
            gid[gb[i]:gb[i + 1]] = i

        xg_groups = {}
        bf_groups = {}

        def load_group(gi):
            if gi < n_groups and gi not in xg_groups:
                lo, hi = gb[gi], gb[gi + 1]
                sz = hi - lo
                bt = bfwp.tile([128, (TGX // 2) * 128], bf16, tag="bfw")
                nc.sync.dma_start(bt[:, :(sz // 2) * 128],
                                  bfT.ap()[:, (lo // 2) * 128:(hi // 2) * 128])
                bf_groups[gi] = bt
                xt = xgwp.tile([128, TGX * 128], bf16, tag="xgw")
                nc.sync.dma_start(xt[:, :sz * 128],
                                  xgT.ap()[:, lo * 128:hi * 128])
                xg_groups[gi] = xt

        def group_tiles(T):
            gi = int(gid[T])
            load_group(gi)
            return xg_groups[gi], bf_groups[gi]

        def bonds_window(w):
            """bonds matmuls (row-tiled pairs) + evac + gate multiply -> cb.

            Tile j (A side, bank 1 cols j*128) runs in PE rows 0-63 while
            tile 4+j (B side, bank 2 cols 512+j*128) runs in rows 64-127."""
            bp = bpsp.tile([128, KPW * 128], f32, tag="bps")   # 2 psum banks
            T0 = w * KPW
            gi = int(gid[T0])
            xt, bt = group_tiles(T0)
            ko = ((T0 - gb[gi]) // 2) * 128         # bf col offset of pair w
            for j in range(4):
                nc.tensor.matmul(bp[:, j * 128:(j + 1) * 128],
                                 lhsT=bt[0:64, ko + j * 128:ko + (j + 1) * 128],
                                 rhs=w3sb[0:64, :],
                                 start=True, stop=True,
                                 tile_position=(0, 0))
                nc.tensor.matmul(bp[:, 512 + j * 128:512 + (j + 1) * 128],
                                 lhsT=bt[64:128, ko + j * 128:ko + (j + 1) * 128],
                                 rhs=w3sb[64:128, :],
                                 start=True, stop=True,
                                 tile_position=(64, 0))
            xt0 = xt
            xo = (T0 - gb[gi]) * 128
            cb = combp.tile([128, KPW * 128], bf16, tag="comb")
            if w % FUSE_MOD == FUSE_MOD - 1:
                # DVE fused: psum * sbuf -> sbuf (one 1024-wide op)
                nc.vector.tensor_tensor(out=cb[:], in0=bp[:],
                                        in1=xt0[:, xo:xo + KPW * 128],
                                        op=mybir.AluOpType.mult)
            else:
                # ACT evacuates psum, DVE multiplies bf16 at 2x
                bs = bsbp.tile([128, KPW * 128], bf16, tag="bsb")
                nc.scalar.copy(bs[:], bp[:])
                nc.vector.tensor_tensor(out=cb[:], in0=bs[:],
                                        in1=xt0[:, xo:xo + KPW * 128],
                                        op=mybir.AluOpType.mult)
            return cb

        cbq = [bonds_window(0)]
        nc.sync.dma_start(patsb[:], pat.ap())
        ob = None
        agg = None
        for w in range(wpc):
            # prefetch xg/bf groups two ahead of consumption
            gi_now = int(gid[min(w * KPW, NT - 1)])
            load_group(gi_now + 1)
            load_group(gi_now + 2)
            if w % 4 == 0:
                agg = aggp.tile([128, 512], f32, tag="agg")
            wb = w % 4
            if w % OG == 0:
                ob = osbp.tile([128, OG * 128], bf16, tag="osb")

            cb = cbq.pop(0)
            if w + 1 < wpc:
                # software pipeline: bonds one pair ahead of scatter(w)
                cbq.append(bonds_window(w + 1))

            # scatter: col-tiled pairs of fixed-pattern matmuls; side A
            # (slots 0-63) and side B (slots 64-127) run concurrently in
            # separate column groups of the PE array
            for t in range(4):
                nc.tensor.matmul(agg[0:64, wb * 128:(wb + 1) * 128],
                                 lhsT=patsb[:, t * 64:(t + 1) * 64],
                                 rhs=cb[:, t * 128:(t + 1) * 128],
                                 start=(t == 0), stop=(t == 3),
                                 tile_position=(0, 0))
                nc.tensor.matmul(agg[64:128, wb * 128:(wb + 1) * 128],
                                 lhsT=patsb[:, t * 64:(t + 1) * 64],
                                 rhs=cb[:, 512 + t * 128:512 + (t + 1) * 128],
                                 start=(t == 0), stop=(t == 3),
                                 tile_position=(0, 64))

            # output add for a finished agg bank
            if wb == 3 or w == wpc - 1:
                nb = wb + 1                      # windows in this bank
                w0 = w - wb                      # first window of bank
                j0 = w0 % OG
                nc.vector.tensor_scalar(
                    ob[:, j0 * 128:(j0 + nb) * 128],
                    agg[:, :nb * 128], 1.0, None,
                    mybir.AluOpType.mult)
            if (w % OG == OG - 1 or w == wpc - 1):
                j = w % OG
                w0 = w - j
                nc.sync.dma_start(out.ap()[:, w0 * 128:(w0 + j + 1) * 128],
                                  ob[:, :(j + 1) * 128])

    nc.compile()
    _prog_cache[key] = nc
    return nc


def _fold_bn(W, b, gamma, beta, mean, var):
    s = (gamma.astype(np.float64) / np.sqrt(var.astype(np.float64) + BN_EPS))
    Wp = W.astype(np.float64) * s[None, :]
    c = (b.astype(np.float64) - mean.astype(np.float64)) * s \
        + beta.astype(np.float64)
    return Wp, c


NS = 64            # slots per (half-)window
CAPTOT = 512       # edge capacity per window (= 4 tiles of 128)


def _make_schedule(deg, n_atoms):
    """Assign atoms (split into chunks of degree <= capmax) to
    (window, slot) so that every 64-slot window has the same
    slot-capacity profile summing to exactly CAPTOT."""
    best = None
    for wpc in (98, 99, 100, 101, 102, 104, 106):
        W = 2 * NCORES * wpc                 # 64-slot windows total
        for capmax in (15, 14, 13):
            nch = np.maximum(1, -(-deg // capmax))       # chunks per atom
            C = int(nch.sum())
            if C > W * NS:
                continue
            reps = nch
            base = deg // reps
            rem = deg - base * reps
            atom_of_chunk = np.repeat(np.arange(n_atoms), reps)
            idx_in_atom = np.arange(C) - np.repeat(
                np.cumsum(reps) - reps, reps)
            cdeg = (np.repeat(base, reps)
                    + (idx_in_atom < np.repeat(rem, reps))).astype(np.int64)
            order = np.argsort(-cdeg, kind="stable")
            cs = cdeg[order]
            nblk = -(-C // W)
            if nblk > NS:
                continue
            prof = np.zeros(NS, np.int64)
            prof[:nblk] = cs[np.arange(nblk) * W]
            S = int(prof.sum())
            if S > CAPTOT:
                continue
            add = CAPTOT - S
            k = 0
            while add > 0:
                prof[k % NS] += 1
                add -= 1
                k += 1
            prof = np.sort(prof)[::-1].copy()
            tiles = NCORES * wpc * KPW
            cand = (tiles, wpc, capmax, prof, order, cs,
                    atom_of_chunk, idx_in_atom, reps)
            if best is None or cand[0] < best[0]:
                best = cand
        if best is not None:
            break
    assert best is not None, "no feasible schedule"
    return best


def _prepare(inputs):
    X = np.asarray(inputs["atom_features"], np.float32)
    BF = np.asarray(inputs["bond_features"], np.float32)
    BP = np.asarray(inputs["bond_pairs"], np.int32)
    n_atoms = X.shape[0]

    W1p, c1 = _fold_bn(np.asarray(inputs["W1"]), np.asarray(inputs["b1"]),
                       np.asarray(inputs["g1"]), np.asarray(inputs["be1"]),
                       np.asarray(inputs["m1"]), np.asarray(inputs["v1"]))
    W2p, c2 = _fold_bn(np.asarray(inputs["W2"]), np.asarray(inputs["b2"]),
                       np.asarray(inputs["g2"]), np.asarray(inputs["be2"]),
                       np.asarray(inputs["m2"]), np.asarray(inputs["v2"]))
    W3p, c3 = _fold_bn(np.asarray(inputs["W3"]), np.asarray(inputs["b3"]),
                       np.asarray(inputs["g3"]), np.asarray(inputs["be3"]),
                       np.asarray(inputs["m3"]), np.asarray(inputs["v3"]))
    W12 = W1p @ W2p
    c12 = c1 @ W2p + c2

    X12 = (X.astype(np.float64) @ W12).astype(np.float32)   # [N, 128]

    dest = BP[:, 0].astype(np.int64)
    src = BP[:, 1].astype(np.int64)

    # sort edges by dest
    perm = np.argsort(dest, kind="stable")
    ds, ss = dest[perm], src[perm]
    bfs = BF[perm]

    deg = np.bincount(ds, minlength=n_atoms).astype(np.int64)

    # host-folded bias terms (incl. atom_h = X@W1p + c1):
    uniq, idxstart = np.unique(ds, return_index=True)
    part_bf = np.add.reduceat(bfs.astype(np.float64), idxstart, axis=0)
    sbsum = np.zeros((n_atoms, BF.shape[1]))
    sbsum[uniq] = part_bf
    part_x = np.add.reduceat(X12[ss].astype(np.float64), idxstart, axis=0)
    sx12 = np.zeros((n_atoms, 128))
    sx12[uniq] = part_x
    Zh = ((sbsum @ W3p) * c12[None, :]
          + deg.astype(np.float64)[:, None] * (c3 * c12)[None, :]
          + sx12 * c3[None, :]
          + X.astype(np.float64) @ W1p + c1[None, :]).astype(np.float32)

    (tiles, wpc, capmax, prof, order, cs, atom_of_chunk,
     idx_in_atom, reps) = _make_schedule(deg, n_atoms)
    W = 2 * NCORES * wpc                    # 64-slot windows
    NT = wpc * KPW
    own = wpc * 128

    # chunk rank r (desc order) -> window r % W, slot position r // W
    C = len(order)
    win_of_chunk = np.empty(C, np.int64)
    slot_of_chunk = np.empty(C, np.int64)
    win_of_chunk[order] = np.arange(C) % W
    slot_of_chunk[order] = np.arange(C) // W

    prof_prefix = np.zeros(NS + 1, np.int64)
    prof_prefix[1:] = np.cumsum(prof)

    # window -> (core, pair, side)
    core_of_win = win_of_chunk // (2 * wpc)
    wloc2 = win_of_chunk - core_of_win * (2 * wpc)
    pair_of_win = wloc2 // 2
    side_of_win = wloc2 % 2
    # instance index within the 64-slot window (0..511)
    inst_in_win = prof_prefix[slot_of_chunk]

    # edge -> chunk mapping: edges of atom a sorted; chunk boundaries at
    # offsets (cumsum of cdeg within atom)
    # chunk edge start (within dest-sorted edge array):
    atom_run_start = np.zeros(n_atoms, np.int64)
    atom_run_start[1:] = np.cumsum(deg)[:-1]
    # cdeg in chunk-id order (cs is rank order)
    cdeg_chunkid = np.empty(C, np.int64)
    cdeg_chunkid[order] = cs
    # offset of chunk within its atom = cumsum of previous chunk degrees
    # chunks of an atom are consecutive chunk ids; use segmented cumsum
    seg_start = np.cumsum(reps) - reps          # first chunk id per atom
    csum = np.cumsum(cdeg_chunkid)
    prev = np.zeros(C, np.int64)
    prev[1:] = csum[:-1]
    atom_first_prev = prev[seg_start]           # cumsum before atom's chunks
    chunk_off_in_atom = prev - np.repeat(atom_first_prev, reps)
    chunk_edge_start = np.repeat(atom_run_start, reps) + chunk_off_in_atom

    # per-edge instance position within window, then global stream pos:
    # tile t of pair p is side A for even t, side B for odd t
    epos = np.empty(len(ds), np.int64)
    nz = cdeg_chunkid > 0
    starts = chunk_edge_start[nz]
    lens = cdeg_chunkid[nz]
    tot = int(lens.sum())
    assert tot == len(ds)
    seg_off = np.repeat(np.cumsum(lens) - lens, lens)
    within = np.arange(tot) - seg_off
    edge_idx = np.repeat(starts, lens) + within
    ii = np.repeat(inst_in_win[nz], lens) + within      # 0..511 in window
    e_core = np.repeat(core_of_win[nz], lens)
    e_pair = np.repeat(pair_of_win[nz], lens)
    e_side = np.repeat(side_of_win[nz], lens)
    e_tile = ii // 128
    e_row = ii - e_tile * 128
    epos[edge_idx] = ((e_core * wpc + e_pair) * KPW
                      + e_side * 4 + e_tile) * 128 + e_row

    TOT = NCORES * NT * 128
    X12b = X12.astype(BF16)
    xgE = np.zeros((TOT, 128), BF16)
    xgE[epos] = X12b[ss]
    bfE = np.zeros((TOT, F_BOND), BF16)
    bfE[epos] = bfs.astype(BF16)

    prim = idx_in_atom == 0
    pos_row = pair_of_win * 128 + side_of_win * 64 + slot_of_chunk

    # fixed patterns: slot id per instance (4 tiles of 128 per window)
    slot_of_inst = np.repeat(np.arange(NS), prof)
    patm = np.zeros((4, 128, NS), np.float32)
    for t in range(4):
        patm[t, np.arange(128), slot_of_inst[t * 128:(t + 1) * 128]] = 1
    pat = np.ascontiguousarray(
        patm.transpose(1, 0, 2).reshape(128, 4 * NS).astype(BF16))

    w3b = np.concatenate([W3p, W3p], axis=0)          # [128, 128]
    consts = dict(w3=np.ascontiguousarray(w3b.astype(BF16)), pat=pat)

    in_maps = []
    for c in range(NCORES):
        sl = slice(c * NT * 128, (c + 1) * NT * 128)
        m = dict(consts)
        m["xgT"] = np.ascontiguousarray(
            xgE[sl].reshape(NT, 128, 128).transpose(1, 0, 2)
            .reshape(128, NT * 128))
        # pack bf features: partitions 0-63 = A tiles (0-3 of each pair),
        # 64-127 = B tiles (4-7), col block j of pair p = bf col p*4+j
        bfc = bfE[sl].reshape(NT // KPW, KPW, 128, F_BOND)
        blocks = np.concatenate(
            [bfc[:, 0:4].transpose(0, 1, 3, 2),
             bfc[:, 4:8].transpose(0, 1, 3, 2)], axis=2)   # [p, 4, 128, 128]
        m["bfT"] = np.ascontiguousarray(
            blocks.transpose(2, 0, 1, 3).reshape(128, (NT // 2) * 128))
        in_maps.append(m)

    # output merge info
    merge = dict(core=core_of_win, row=pos_row, atom=atom_of_chunk,
                 prim=prim, wpc=wpc, prof=tuple(int(x) for x in prof),
                 Zh=Zh)
    return in_maps, merge


def run(inputs):
    global LAST_RESULTS
    in_maps, merge = _prepare(inputs)
    wpc = merge["wpc"]
    nc = _build_program(wpc, merge["prof"])
    res = run_bass_kernel_spmd(nc, in_maps, core_ids=list(range(NCORES)),
                               trace=TRACE)
    LAST_RESULTS = res
    own = wpc * 128
    od = np.stack([res.results[c]["out"].astype(np.float32)
                   .reshape(128, wpc, 128).transpose(1, 0, 2)
                   .reshape(own, 128)
                   for c in range(NCORES)])        # [8, own, 128]
    n_atoms = N
    out = np.zeros((n_atoms, 128), np.float32)
    core, row, atom, prim = (merge["core"], merge["row"], merge["atom"],
                             merge["prim"])
    out[atom[prim]] = od[core[prim], row[prim]]
    sec = ~prim
    if sec.any():
        np.add.at(out, atom[sec], od[core[sec], row[sec]])
    out += merge["Zh"]
    return out


def kernel(**inputs):
    return run(inputs)
